# revision 38
# baseline (speedup 1.0000x reference)
"""Trainium2 Bass kernel for single-head attention (B=8, N=3136, C=147, D=64).

Sharding: data-parallel over batch across 8 NeuronCores (1 batch element/core).

The device runs only the O(N^2) attention core (S = q@k^T, exp, P@V) --
>99% of the FLOPs; the QKV projections and the epilogue (softmax
normalization, W_proj, bias, v-residual) run on the host in fp32.

Structure (HW-trace-driven; 88.8us baseline -> ~80us):
  - S^T is 2x ROW-TILED (64x128 array mode): D=64 means the S^T
    contraction is only 64 partitions, so row tiles (0,0)/(64,0) compute
    TWO j-tiles concurrently in one 448-col stream (1.83x microbenched
    vs serial K=128 dup): tile A does even j-tiles from kT2[0:64,:],
    tile B odd j-tiles from kT2[64:128,:], both streaming q from their
    SBUF partition half. kT is zero-padded to 3200 cols so all 25
    j-tiles are 128 wide (padded cols give exp(0)=1 times zero-padded
    v rows = 0).
  - PV is ALSO row-tiled (each j-tile's K=128 contraction splits into
    h0/h64 halves accumulating into two PSUM banks, summed during the
    evacuation): MIXING full-array matmuls with row-tiled pairs breaks
    LDWEIGHTS prefetch (weight-buffer conflicts; measured 793ns/slot vs
    the 567 ideal). Homogeneous row-group streams keep every LDW
    prefetched (measured 189ns/stream pitch).
  - exp is the near-bottleneck (~7.2us/chunk vs PE ~7.4): ACT (1
    elem/cyc @1.2GHz hardware exp, 6 dual-tiles + the odd single) and
    DVE (1 elem/cyc @0.96GHz Schraudolph bit trick: i16 = round(s*EA +
    EB) in one tensor_scalar whose bits ARE bf16 ~= e^s; 6 dual-tiles)
    split the 13 exp units per 448-wide i-chunk; ACT also copies o_B to
    SBUF and DVE adds o_A to it for the output DMA.
  - PVs trail their exp by 3 slots: the exp latency chain (S^T end +
    sem + ~1050ns + sem) overruns a 2-slot deadline by ~220ns, which
    inflated every slot. PSUM: st pairs 2 banks x 3 bufs + o pair
    2 banks x 1 buf = 8 banks; the previous chunk's PVs retire at slot
    0-1 of the next chunk and its o banks evacuate before the new
    chunk's first PV (slot 3) clears them.
  - Chunk 0 is input-DMA-paced (~80 GB/s per queue), so it uses the
    dense full-array dup form (one K=128 matmul per j-tile computing 2S
    on the duplicated halves; the 0.5 folds into the exp affine) to keep
    the PE continuously busy while inputs land and the HAM clock gate
    opens (PE idle gaps let HAM re-throttle 2.4->1.2GHz). Inputs stream
    as fine-grained dma_start pieces (per-piece completion sems) across
    three queues, ordered by first use.

Avoided dead ends (all measured on HW): PV row-splitting does NOT save
PE time by itself (stream columns are invariant; the win is only LDW
prefetch homogeneity); transposed PV (P as weights) is LDW-bound;
splitting the evacuation into two DMA'd halves summed on the host is
slower; small first/last chunks lose more to per-chunk exp instruction
overheads (~260ns fixed per ACT op) than they save; scheduler-visible
junk gap-fillers into live PSUM banks get reordered and corrupt the
accumulators; the end-of-NEFF semaphore sweep (~250 sems, ~9us) is
framework-fixed.
"""
import sys

for _p in ("/opt/trn_rl_repo",):
    if _p not in sys.path:
        sys.path.append(_p)

import numpy as np
import ml_dtypes
from contextlib import ExitStack

import concourse.bass as bass
import concourse.bacc as bacc
import concourse.tile as tile
from concourse import mybir
from concourse.bass_utils import run_bass_kernel_spmd

P = 128
SEQ = 3136        # N
CH = 147          # C
D = 64            # head dim
SCALE = D ** -0.5
NT = 25           # j-tiles (kT zero-padded 3136 -> 3200 = 25*128)
KPAD = NT * P     # 3200
IC = 448          # i-chunk width (3136 = 7*448)
F32 = mybir.dt.float32
BF = mybir.dt.bfloat16
I16 = mybir.dt.int16
EXP = mybir.ActivationFunctionType.Exp
ADD = mybir.AluOpType.add

# Schraudolph constants: i16 = round(s * EA + EB); bits read as bf16 give
# ~e^s * (1 + eps(frac)), EB calibrated so E[eps] ~= 0.
EA = 128.0 * 1.4426950408889634
EB = 127.0 * 128.0 - 7.37

# exp engine assignment per slot (13 slots: 12 dual j-tile + 1 single):
# DVE (slower: 1090ns vs 1008ns per dual, plus the o add) takes 6 duals;
# ACT takes 6 duals + the single + the o_B evacuation copy.
DVE_SLOTS = frozenset({0, 2, 4, 6, 8, 10})

_cache = {}


def _ichunks():
    out = []
    i0 = 0
    while i0 < SEQ:
        out.append((i0, min(IC, SEQ - i0)))
        i0 += IC
    return out


def build():
    nc = bacc.Bacc("TRN2", target_bir_lowering=False, debug=False, num_devices=8)
    qT2d = nc.declare_dram_parameter("qT2", [P, SEQ], BF, isOutput=False)
    kT2d = nc.declare_dram_parameter("kT2", [P, KPAD], BF, isOutput=False)
    v_aug = nc.declare_dram_parameter("v_aug", [P, NT, D + 1], BF, isOutput=False)
    chunks = _ichunks()
    oT = nc.declare_dram_parameter("oT", [len(chunks), D + 1, IC], F32,
                                   isOutput=True)

    with ExitStack() as ctx:
        tc = ctx.enter_context(tile.TileContext(nc))
        singles = ctx.enter_context(tc.tile_pool(name="singles", bufs=1))

        qT2 = singles.tile([P, SEQ], BF)   # qT duplicated in both halves
        kT2 = singles.tile([P, KPAD], BF)  # kT duplicated in both halves
        va = singles.tile([P, NT, D + 1], BF)
        junk_w = singles.tile([P, P], BF)
        junk_x = singles.tile([P, 256], BF)
        junk_e = singles.tile([P, 8], F32)
        junk_p = singles.tile([P, 8], BF)
        # Junk init FIRST on GpSimd so the HAM warm-up matmuls can start
        # immediately (memsets queued behind DMA issues delay the first
        # matmul and let HAM re-throttle).
        nc.gpsimd.memset(junk_w, 0.5)
        nc.gpsimd.memset(junk_x, 0.5)
        nc.gpsimd.memset(junk_e, 0.5)
        # Input DMAs: chunk 0 needs all of kT + q[0:448] + all of va. Each
        # dma_start is a separate completion sem, so FINE-GRAINED pieces
        # release their consumers as they land (one big piece would gate
        # slot 2 on the last byte of slot 25's tile). Per-queue effective
        # bandwidth is ~80 GB/s; chunk 0's dense matmuls consume kT at
        # ~87 GB/s, so kT is fed from two queues and q/va from the others,
        # ordered by first-use time.
        nc.gpsimd.dma_start(out=kT2[:, 0:128], in_=kT2d[:, 0:128])
        nc.scalar.dma_start(out=qT2[:, 0:IC], in_=qT2d[:, 0:IC])
        nc.sync.dma_start(out=kT2[:, 1536:2048], in_=kT2d[:, 1536:2048])
        nc.gpsimd.dma_start(out=va[:, 0:8, :], in_=v_aug[:, 0:8, :])
        nc.scalar.dma_start(out=kT2[:, 128:512], in_=kT2d[:, 128:512])
        nc.scalar.dma_start(out=kT2[:, 512:1024], in_=kT2d[:, 512:1024])
        nc.scalar.dma_start(out=kT2[:, 1024:1536], in_=kT2d[:, 1024:1536])
        nc.gpsimd.dma_start(out=va[:, 8:NT, :], in_=v_aug[:, 8:NT, :])
        nc.sync.dma_start(out=kT2[:, 2048:2560], in_=kT2d[:, 2048:2560])
        nc.sync.dma_start(out=kT2[:, 2560:KPAD], in_=kT2d[:, 2560:KPAD])
        for (n0, csz) in chunks[1:]:
            nc.sync.dma_start(out=qT2[:, n0:n0 + csz],
                              in_=qT2d[:, n0:n0 + csz])

        # HAM pre-warm + ACT exp-table preload, overlapping the input DMA.
        # Full-array mode, same as chunk 0's dense form (no mode drain).
        with ExitStack() as wctx:
            warm_ps = wctx.enter_context(
                tc.tile_pool(name="warm_ps", bufs=2, space="PSUM"))
            nc.scalar.activation(junk_p, junk_e, EXP)
            for i in range(11):
                wp = warm_ps.tile([P, 2, 512], F32, name="warm")
                nc.tensor.matmul(wp[:, 0, 0:256], junk_w, junk_x,
                                 start=True, stop=True)

        # ---------------- attention ----------------
        with ExitStack() as cctx:
            st_ps = cctx.enter_context(tc.tile_pool(name="st_ps", bufs=3, space="PSUM"))
            o_ps_pool = cctx.enter_context(tc.tile_pool(name="o_ps", bufs=1, space="PSUM"))
            p_pool = cctx.enter_context(tc.tile_pool(name="p_sb", bufs=6))
            o_sb_pool = cctx.enter_context(tc.tile_pool(name="o_sb", bufs=2))
            ob_sb_pool = cctx.enter_context(tc.tile_pool(name="ob_sb", bufs=2))
            nslots = (NT + 1) // 2    # 13: 12 dual + 1 single

            def emit_pv(p, pt, o_ps, icsz, dense):
                for s in (0, 1):
                    jt = 2 * pt + s
                    if jt >= NT:
                        break
                    if True:
                        nc.tensor.matmul(o_ps[:, 0, 0:icsz], va[0:64, jt, :],
                                         p[0:64, s, 0:icsz],
                                         start=(jt == 0), stop=(jt == NT - 1),
                                         tile_position=(0, 0))
                        # j-tile 24's h64 rows are all zero-padding in va,
                        # so its PV-B contributes exactly 0 -- skip it (the
                        # B chain therefore stops at jt NT-2).
                        if jt < NT - 1:
                            nc.tensor.matmul(o_ps[:, 1, 0:icsz],
                                             va[64:128, jt, :],
                                             p[64:128, s, 0:icsz],
                                             start=(jt == 0),
                                             stop=(jt == NT - 2),
                                             tile_position=(64, 0))

            def emit_evac(o_ps, ci, icsz, dense):
                osb = o_sb_pool.tile([D + 1, IC], F32, name="osb")
                ob = ob_sb_pool.tile([D + 1, IC], F32, name="ob")
                nc.scalar.copy(ob[:, 0:icsz], o_ps[:, 1, 0:icsz])
                nc.vector.scalar_tensor_tensor(
                    out=osb[:, 0:icsz], in0=o_ps[:, 0, 0:icsz],
                    scalar=0.0, in1=ob[:, 0:icsz], op0=ADD, op1=ADD)
                nc.gpsimd.dma_start(out=oT[ci, :, 0:icsz], in_=osb[:, 0:icsz])

            pend = []            # (p, pt, o_ps, icsz, dense)
            pending_evac = None  # (o_ps, chunk index, icsz, dense)
            for ci, (i0, icsz) in enumerate(chunks):
                dense = ci == 0
                o_ps = o_ps_pool.tile([D + 1, 2, 512], F32, name="o")
                for pt in range(nslots):
                    jtA, jtB = 2 * pt, 2 * pt + 1
                    st = st_ps.tile([P, 2, 512], F32, name="st")
                    p = p_pool.tile([P, 2, IC], BF, name="p")
                    nc.tensor.matmul(
                        st[:, 0, 0:icsz],
                        kT2[0:64, jtA * P:(jtA + 1) * P],
                        qT2[0:64, i0:i0 + icsz],
                        start=True, stop=True, tile_position=(0, 0))
                    if jtB < NT:
                        if dense:
                            # Chunk 0 is DMA-paced: run its second S^T on
                            # the SAME row tile (serial, half power, smooth
                            # ~87GB/s kT consumption) instead of the
                            # concurrent (64,0) tile, which would burst at
                            # 2x the DMA feed rate and idle-gap the PE
                            # (HAM re-throttle).
                            nc.tensor.matmul(
                                st[:, 1, 0:icsz],
                                kT2[0:64, jtB * P:(jtB + 1) * P],
                                qT2[0:64, i0:i0 + icsz],
                                start=True, stop=True, tile_position=(0, 0))
                        else:
                            nc.tensor.matmul(
                                st[:, 1, 0:icsz],
                                kT2[64:128, jtB * P:(jtB + 1) * P],
                                qT2[64:128, i0:i0 + icsz],
                                start=True, stop=True, tile_position=(64, 0))
                        if pt in DVE_SLOTS:
                            nc.vector.tensor_scalar(
                                out=p[:, :, 0:icsz].bitcast(I16),
                                in0=st[:, :, 0:icsz],
                                scalar1=EA, scalar2=EB,
                                op0=mybir.AluOpType.mult,
                                op1=mybir.AluOpType.add)
                        else:
                            nc.scalar.activation(p[:, :, 0:icsz],
                                                 st[:, :, 0:icsz], EXP)
                    else:
                        nc.scalar.activation(p[:, 0, 0:icsz],
                                             st[:, 0, 0:icsz], EXP)
                    pend.append((p, pt, o_ps, icsz, dense))
                    # PVs trail by 3 slots mid-chunk: the exp latency chain
                    # (S^T end + sem + ~1050ns exp + sem) overruns a 2-slot
                    # deadline by ~220ns, which inflated every slot (v5
                    # post-mortem). Trail 1 at the first two slots so the
                    # previous chunk's last PVs retire early; its o banks
                    # are evacuated right after (parallel ACT/DVE copies,
                    # ~630ns) before this chunk's first PV (slot 3+) clears
                    # them (o pool is single-buffered).
                    trail = 1 if pt <= 1 else 3
                    while len(pend) > trail:
                        emit_pv(*pend.pop(0))
                    if pt == 0 and pending_evac is not None:
                        emit_evac(*pending_evac)
                        pending_evac = None
                pending_evac = (o_ps, ci, icsz, dense)
            while pend:
                emit_pv(*pend.pop(0))
            emit_evac(*pending_evac)

    nc.compile()
    return nc


def prep_in_maps(x, W_qkv, W_proj, b_proj):
    """Host-side prep: per-core transposed/duplicated bf16 operand layouts."""
    B = x.shape[0]
    bf = ml_dtypes.bfloat16
    Wq = (W_qkv[:, 0:D] * SCALE).astype(np.float32)
    Wk = W_qkv[:, D:2 * D].astype(np.float32)
    Wv = W_qkv[:, 2 * D:3 * D].astype(np.float32)
    in_maps = []
    vs = []
    for b in range(B):
        xb = x[b].astype(np.float32)
        v = xb @ Wv                                  # [N, D] fp32
        vs.append(v)
        vpad = np.zeros((KPAD, D + 1), np.float32)
        vpad[0:SEQ, 0:D] = v
        vpad[0:SEQ, D] = 1.0
        va = np.ascontiguousarray(
            vpad.reshape(NT, P, D + 1).transpose(1, 0, 2)).astype(bf)
        qT = np.ascontiguousarray((xb @ Wq).T)       # [D, N], pre-scaled
        kTp = np.zeros((D, KPAD), np.float32)
        kTp[:, 0:SEQ] = (xb @ Wk).T
        in_maps.append({
            "qT2": np.concatenate([qT, qT], axis=0).astype(bf),
            "kT2": np.concatenate([kTp, kTp], axis=0).astype(bf),
            "v_aug": va,
        })
    return in_maps, vs


def postprocess(results, vs, W_proj, b_proj):
    B = len(vs)
    chunks = _ichunks()
    out = np.empty((B, SEQ, D), np.float32)
    Wp = W_proj.astype(np.float32)
    bp = b_proj.astype(np.float32)
    for b in range(B):
        oT = results[b]["oT"]                        # [NCHUNK, 65, IC]
        O = np.concatenate(
            [oT[ci, :, 0:csz] for ci, (_, csz) in enumerate(chunks)], axis=1)
        attn = (O[0:D] / O[D:D + 1]).T               # [N, D]
        out[b] = vs[b] + attn @ Wp + bp
    return out


def kernel(x, W_qkv, W_proj, b_proj):
    B = x.shape[0]
    if "nc" not in _cache:
        _cache["nc"] = build()
    nc = _cache["nc"]
    in_maps, vs = prep_in_maps(x, W_qkv, W_proj, b_proj)
    res = run_bass_kernel_spmd(nc, in_maps, core_ids=list(range(B)))
    return postprocess(res.results, vs, W_proj, b_proj)


if __name__ == "__main__":
    rng = np.random.default_rng(0)
    x = rng.standard_normal((8, SEQ, CH), dtype=np.float32)
    W_qkv = (rng.standard_normal((CH, 3 * D), dtype=np.float32) * CH ** -0.5)
    W_proj = (rng.standard_normal((D, D), dtype=np.float32) * D ** -0.5)
    b_proj = np.zeros(D, dtype=np.float32)
    out = kernel(x, W_qkv, W_proj, b_proj)
    print("out", out.shape, out.dtype)


# revision 39
# speedup vs baseline: 1.0374x; 1.0374x over previous
"""Trainium2 Bass kernel for single-head attention (B=8, N=3136, C=147, D=64).

Sharding: data-parallel over batch across 8 NeuronCores (1 batch element/core).

The device runs only the O(N^2) attention core (S = q@k^T, exp, P@V) --
>99% of the FLOPs; the QKV projections and the epilogue (softmax
normalization, W_proj, bias, v-residual) run on the host in fp32.

Structure (HW-trace-driven; 88.8us baseline -> ~80us):
  - S^T is 2x ROW-TILED (64x128 array mode): D=64 means the S^T
    contraction is only 64 partitions, so row tiles (0,0)/(64,0) compute
    TWO j-tiles concurrently in one 448-col stream (1.83x microbenched
    vs serial K=128 dup): tile A does even j-tiles from kT2[0:64,:],
    tile B odd j-tiles from kT2[64:128,:], both streaming q from their
    SBUF partition half. kT is zero-padded to 3200 cols so all 25
    j-tiles are 128 wide (padded cols give exp(0)=1 times zero-padded
    v rows = 0).
  - PV is ALSO row-tiled (each j-tile's K=128 contraction splits into
    h0/h64 halves accumulating into two PSUM banks, summed during the
    evacuation): MIXING full-array matmuls with row-tiled pairs breaks
    LDWEIGHTS prefetch (weight-buffer conflicts; measured 793ns/slot vs
    the 567 ideal). Homogeneous row-group streams keep every LDW
    prefetched (measured 189ns/stream pitch).
  - exp is the near-bottleneck (~7.2us/chunk vs PE ~7.4): ACT (1
    elem/cyc @1.2GHz hardware exp, 6 dual-tiles + the odd single) and
    DVE (1 elem/cyc @0.96GHz Schraudolph bit trick: i16 = round(s*EA +
    EB) in one tensor_scalar whose bits ARE bf16 ~= e^s; 6 dual-tiles)
    split the 13 exp units per 448-wide i-chunk; ACT also copies o_B to
    SBUF and DVE adds o_A to it for the output DMA.
  - PVs trail their exp by 3 slots: the exp latency chain (S^T end +
    sem + ~1050ns + sem) overruns a 2-slot deadline by ~220ns, which
    inflated every slot. PSUM: st pairs 2 banks x 3 bufs + o pair
    2 banks x 1 buf = 8 banks; the previous chunk's PVs retire at slot
    0-1 of the next chunk and its o banks evacuate before the new
    chunk's first PV (slot 3) clears them.
  - Chunk 0 is input-DMA-paced (~80 GB/s per queue), so it uses the
    dense full-array dup form (one K=128 matmul per j-tile computing 2S
    on the duplicated halves; the 0.5 folds into the exp affine) to keep
    the PE continuously busy while inputs land and the HAM clock gate
    opens (PE idle gaps let HAM re-throttle 2.4->1.2GHz). Inputs stream
    as fine-grained dma_start pieces (per-piece completion sems) across
    three queues, ordered by first use.

Avoided dead ends (all measured on HW): PV row-splitting does NOT save
PE time by itself (stream columns are invariant; the win is only LDW
prefetch homogeneity); transposed PV (P as weights) is LDW-bound;
splitting the evacuation into two DMA'd halves summed on the host is
slower; small first/last chunks lose more to per-chunk exp instruction
overheads (~260ns fixed per ACT op) than they save; scheduler-visible
junk gap-fillers into live PSUM banks get reordered and corrupt the
accumulators; the end-of-NEFF semaphore sweep (~250 sems, ~9us) is
framework-fixed.
"""
import sys

for _p in ("/opt/trn_rl_repo",):
    if _p not in sys.path:
        sys.path.append(_p)

import numpy as np
import ml_dtypes
from contextlib import ExitStack

import concourse.bass as bass
import concourse.bacc as bacc
import concourse.tile as tile
from concourse import mybir
from concourse.bass_utils import run_bass_kernel_spmd

P = 128
SEQ = 3136        # N
CH = 147          # C
D = 64            # head dim
SCALE = D ** -0.5
NT = 25           # j-tiles (kT zero-padded 3136 -> 3200 = 25*128)
KPAD = NT * P     # 3200
IC = 448          # i-chunk width (3136 = 7*448)
F32 = mybir.dt.float32
BF = mybir.dt.bfloat16
I16 = mybir.dt.int16
EXP = mybir.ActivationFunctionType.Exp
ADD = mybir.AluOpType.add

# Schraudolph constants: i16 = round(s * EA + EB); bits read as bf16 give
# ~e^s * (1 + eps(frac)), EB calibrated so E[eps] ~= 0.
EA = 128.0 * 1.4426950408889634
EB = 127.0 * 128.0 - 7.37

# exp engine assignment per slot (13 slots: 12 dual j-tile + 1 single):
# DVE (slower: 1090ns vs 1008ns per dual, plus the o add) takes 6 duals;
# ACT takes 6 duals + the single + the o_B evacuation copy.
DVE_SLOTS = frozenset({0, 2, 4, 6, 8, 10})

_cache = {}


def _ichunks():
    out = []
    i0 = 0
    while i0 < SEQ:
        out.append((i0, min(IC, SEQ - i0)))
        i0 += IC
    return out


def build():
    nc = bacc.Bacc("TRN2", target_bir_lowering=False, debug=False, num_devices=8)
    qT2d = nc.declare_dram_parameter("qT2", [P, SEQ], BF, isOutput=False)
    kT2d = nc.declare_dram_parameter("kT2", [P, KPAD], BF, isOutput=False)
    v_aug = nc.declare_dram_parameter("v_aug", [P, NT, D + 1], BF, isOutput=False)
    chunks = _ichunks()
    oT = nc.declare_dram_parameter("oT", [len(chunks), D + 1, IC], F32,
                                   isOutput=True)

    with ExitStack() as ctx:
        tc = ctx.enter_context(tile.TileContext(nc))
        singles = ctx.enter_context(tc.tile_pool(name="singles", bufs=1))

        qT2 = singles.tile([P, SEQ], BF)   # qT duplicated in both halves
        kT2 = singles.tile([P, KPAD], BF)  # kT duplicated in both halves
        va = singles.tile([P, NT, D + 1], BF)
        junk_w = singles.tile([P, P], BF)
        junk_x = singles.tile([P, 256], BF)
        junk_e = singles.tile([P, 8], F32)
        junk_p = singles.tile([P, 8], BF)
        # Junk init FIRST on GpSimd so the HAM warm-up matmuls can start
        # immediately (memsets queued behind DMA issues delay the first
        # matmul and let HAM re-throttle).
        nc.gpsimd.memset(junk_w, 0.5)
        nc.gpsimd.memset(junk_x, 0.5)
        nc.gpsimd.memset(junk_e, 0.5)
        # Input DMAs: chunk 0 needs all of kT + q[0:448] + all of va. Each
        # dma_start is a separate completion sem, so FINE-GRAINED pieces
        # release their consumers as they land (one big piece would gate
        # slot 2 on the last byte of slot 25's tile). Per-queue effective
        # bandwidth is ~80 GB/s; chunk 0's dense matmuls consume kT at
        # ~87 GB/s, so kT is fed from two queues and q/va from the others,
        # ordered by first-use time.
        nc.gpsimd.dma_start(out=kT2[:, 0:128], in_=kT2d[:, 0:128])
        nc.scalar.dma_start(out=qT2[:, 0:IC], in_=qT2d[:, 0:IC])
        nc.sync.dma_start(out=kT2[:, 1536:2048], in_=kT2d[:, 1536:2048])
        nc.gpsimd.dma_start(out=va[:, 0:8, :], in_=v_aug[:, 0:8, :])
        nc.scalar.dma_start(out=kT2[:, 128:512], in_=kT2d[:, 128:512])
        nc.scalar.dma_start(out=kT2[:, 512:1024], in_=kT2d[:, 512:1024])
        nc.scalar.dma_start(out=kT2[:, 1024:1536], in_=kT2d[:, 1024:1536])
        nc.gpsimd.dma_start(out=va[:, 8:NT, :], in_=v_aug[:, 8:NT, :])
        nc.sync.dma_start(out=kT2[:, 2048:2560], in_=kT2d[:, 2048:2560])
        nc.sync.dma_start(out=kT2[:, 2560:KPAD], in_=kT2d[:, 2560:KPAD])
        for (n0, csz) in chunks[1:]:
            nc.sync.dma_start(out=qT2[:, n0:n0 + csz],
                              in_=qT2d[:, n0:n0 + csz])

        # HAM pre-warm + ACT exp-table preload, overlapping the input DMA.
        # Full-array mode, same as chunk 0's dense form (no mode drain).
        with ExitStack() as wctx:
            warm_ps = wctx.enter_context(
                tc.tile_pool(name="warm_ps", bufs=2, space="PSUM"))
            nc.scalar.activation(junk_p, junk_e, EXP)
            for i in range(11):
                wp = warm_ps.tile([P, 2, 512], F32, name="warm")
                nc.tensor.matmul(wp[:, 0, 0:256], junk_w, junk_x,
                                 start=True, stop=True)

        # ---------------- attention ----------------
        with ExitStack() as cctx:
            st_ps = cctx.enter_context(tc.tile_pool(name="st_ps", bufs=3, space="PSUM"))
            o_ps_pool = cctx.enter_context(tc.tile_pool(name="o_ps", bufs=1, space="PSUM"))
            p_pool = cctx.enter_context(tc.tile_pool(name="p_sb", bufs=6))
            o_sb_pool = cctx.enter_context(tc.tile_pool(name="o_sb", bufs=2))
            ob_sb_pool = cctx.enter_context(tc.tile_pool(name="ob_sb", bufs=2))
            nslots = (NT + 1) // 2    # 13: 12 dual + 1 single

            def emit_pv(p, pt, o_ps, icsz, dense):
                for s in (0, 1):
                    jt = 2 * pt + s
                    if jt >= NT:
                        break
                    if dense:
                        nc.tensor.matmul(o_ps[:, 0, 0:icsz], va[:, jt, :],
                                         p[:, s, 0:icsz],
                                         start=(jt == 0), stop=(jt == NT - 1))
                    else:
                        nc.tensor.matmul(o_ps[:, 0, 0:icsz], va[0:64, jt, :],
                                         p[0:64, s, 0:icsz],
                                         start=(jt == 0), stop=(jt == NT - 1),
                                         tile_position=(0, 0))
                        # j-tile 24's h64 rows are all zero-padding in va,
                        # so its PV-B contributes exactly 0 -- skip it (the
                        # B chain therefore stops at jt NT-2).
                        if jt < NT - 1:
                            nc.tensor.matmul(o_ps[:, 1, 0:icsz],
                                             va[64:128, jt, :],
                                             p[64:128, s, 0:icsz],
                                             start=(jt == 0),
                                             stop=(jt == NT - 2),
                                             tile_position=(64, 0))

            def emit_evac(o_ps, ci, icsz, dense):
                osb = o_sb_pool.tile([D + 1, IC], F32, name="osb")
                if dense:
                    nc.vector.tensor_copy(osb[:, 0:icsz], o_ps[:, 0, 0:icsz])
                else:
                    ob = ob_sb_pool.tile([D + 1, IC], F32, name="ob")
                    nc.scalar.copy(ob[:, 0:icsz], o_ps[:, 1, 0:icsz])
                    nc.vector.scalar_tensor_tensor(
                        out=osb[:, 0:icsz], in0=o_ps[:, 0, 0:icsz],
                        scalar=0.0, in1=ob[:, 0:icsz], op0=ADD, op1=ADD)
                nc.gpsimd.dma_start(out=oT[ci, :, 0:icsz], in_=osb[:, 0:icsz])

            pend = []            # (p, pt, o_ps, icsz, dense)
            pending_evac = None  # (o_ps, chunk index, icsz, dense)
            for ci, (i0, icsz) in enumerate(chunks):
                dense = ci == 0
                esc = 0.5 if dense else 1.0
                o_ps = o_ps_pool.tile([D + 1, 2, 512], F32, name="o")
                for pt in range(nslots):
                    jtA, jtB = 2 * pt, 2 * pt + 1
                    st = st_ps.tile([P, 2, 512], F32, name="st")
                    p = p_pool.tile([P, 2, IC], BF, name="p")
                    if dense:
                        nc.tensor.matmul(
                            st[:, 0, 0:icsz],
                            kT2[:, jtA * P:(jtA + 1) * P],
                            qT2[:, i0:i0 + icsz],
                            start=True, stop=True)
                    else:
                        nc.tensor.matmul(
                            st[:, 0, 0:icsz],
                            kT2[0:64, jtA * P:(jtA + 1) * P],
                            qT2[0:64, i0:i0 + icsz],
                            start=True, stop=True, tile_position=(0, 0))
                    if jtB < NT:
                        if dense:
                            nc.tensor.matmul(
                                st[:, 1, 0:icsz],
                                kT2[:, jtB * P:(jtB + 1) * P],
                                qT2[:, i0:i0 + icsz],
                                start=True, stop=True)
                        else:
                            nc.tensor.matmul(
                                st[:, 1, 0:icsz],
                                kT2[64:128, jtB * P:(jtB + 1) * P],
                                qT2[64:128, i0:i0 + icsz],
                                start=True, stop=True, tile_position=(64, 0))
                        if pt in DVE_SLOTS:
                            nc.vector.tensor_scalar(
                                out=p[:, :, 0:icsz].bitcast(I16),
                                in0=st[:, :, 0:icsz],
                                scalar1=EA * esc, scalar2=EB,
                                op0=mybir.AluOpType.mult,
                                op1=mybir.AluOpType.add)
                        else:
                            nc.scalar.activation(p[:, :, 0:icsz],
                                                 st[:, :, 0:icsz], EXP,
                                                 scale=esc)
                    else:
                        nc.scalar.activation(p[:, 0, 0:icsz],
                                             st[:, 0, 0:icsz], EXP, scale=esc)
                    pend.append((p, pt, o_ps, icsz, dense))
                    # PVs trail by 3 slots mid-chunk: the exp latency chain
                    # (S^T end + sem + ~1050ns exp + sem) overruns a 2-slot
                    # deadline by ~220ns, which inflated every slot (v5
                    # post-mortem). Trail 1 at the first two slots so the
                    # previous chunk's last PVs retire early; its o banks
                    # are evacuated right after (parallel ACT/DVE copies,
                    # ~630ns) before this chunk's first PV (slot 3+) clears
                    # them (o pool is single-buffered).
                    trail = 1 if pt <= 1 else 3
                    while len(pend) > trail:
                        emit_pv(*pend.pop(0))
                    if pt == 0 and pending_evac is not None:
                        emit_evac(*pending_evac)
                        pending_evac = None
                pending_evac = (o_ps, ci, icsz, dense)
            while pend:
                emit_pv(*pend.pop(0))
            emit_evac(*pending_evac)

    nc.compile()
    return nc


def prep_in_maps(x, W_qkv, W_proj, b_proj):
    """Host-side prep: per-core transposed/duplicated bf16 operand layouts."""
    B = x.shape[0]
    bf = ml_dtypes.bfloat16
    Wq = (W_qkv[:, 0:D] * SCALE).astype(np.float32)
    Wk = W_qkv[:, D:2 * D].astype(np.float32)
    Wv = W_qkv[:, 2 * D:3 * D].astype(np.float32)
    in_maps = []
    vs = []
    for b in range(B):
        xb = x[b].astype(np.float32)
        v = xb @ Wv                                  # [N, D] fp32
        vs.append(v)
        vpad = np.zeros((KPAD, D + 1), np.float32)
        vpad[0:SEQ, 0:D] = v
        vpad[0:SEQ, D] = 1.0
        va = np.ascontiguousarray(
            vpad.reshape(NT, P, D + 1).transpose(1, 0, 2)).astype(bf)
        qT = np.ascontiguousarray((xb @ Wq).T)       # [D, N], pre-scaled
        kTp = np.zeros((D, KPAD), np.float32)
        kTp[:, 0:SEQ] = (xb @ Wk).T
        in_maps.append({
            "qT2": np.concatenate([qT, qT], axis=0).astype(bf),
            "kT2": np.concatenate([kTp, kTp], axis=0).astype(bf),
            "v_aug": va,
        })
    return in_maps, vs


def postprocess(results, vs, W_proj, b_proj):
    B = len(vs)
    chunks = _ichunks()
    out = np.empty((B, SEQ, D), np.float32)
    Wp = W_proj.astype(np.float32)
    bp = b_proj.astype(np.float32)
    for b in range(B):
        oT = results[b]["oT"]                        # [NCHUNK, 65, IC]
        O = np.concatenate(
            [oT[ci, :, 0:csz] for ci, (_, csz) in enumerate(chunks)], axis=1)
        attn = (O[0:D] / O[D:D + 1]).T               # [N, D]
        out[b] = vs[b] + attn @ Wp + bp
    return out


def kernel(x, W_qkv, W_proj, b_proj):
    B = x.shape[0]
    if "nc" not in _cache:
        _cache["nc"] = build()
    nc = _cache["nc"]
    in_maps, vs = prep_in_maps(x, W_qkv, W_proj, b_proj)
    res = run_bass_kernel_spmd(nc, in_maps, core_ids=list(range(B)))
    return postprocess(res.results, vs, W_proj, b_proj)


if __name__ == "__main__":
    rng = np.random.default_rng(0)
    x = rng.standard_normal((8, SEQ, CH), dtype=np.float32)
    W_qkv = (rng.standard_normal((CH, 3 * D), dtype=np.float32) * CH ** -0.5)
    W_proj = (rng.standard_normal((D, D), dtype=np.float32) * D ** -0.5)
    b_proj = np.zeros(D, dtype=np.float32)
    out = kernel(x, W_qkv, W_proj, b_proj)
    print("out", out.shape, out.dtype)
